# revision 15
# baseline (speedup 1.0000x reference)
"""Additive (Bahdanau) attention kernel for Trainium2, data-parallel over 8 NeuronCores.

Computation (per batch b):
    q_proj = query @ Wq.T + bq                  # [H]
    m_proj = memory[b] @ Wm.T                   # [M, H]
    hidden = tanh(q_proj + m_proj)              # [M, H]
    attn   = hidden @ v                         # [M]
    w      = softmax(attn)                      # [M]
    wmem   = w @ memory[b]                      # [D]

Layout strategy (per core, 8 local batches):
  - m_proj computed transposed: [h on partitions, m free], so the q_proj add +
    tanh fuse into one ACT instruction (per-partition bias), and the v-reduction
    is a PE matmul with v as the (1-column) stationary operand.
  - memory tiles are cast to bf16 and transposed to d-major via the DMA XBAR
    (fp32 has no XBAR path; bf16 matmul streams at 1 cycle/row vs 4 for fp32).
  - softmax without max-subtraction (attn is bounded by sum|v| ~ 25), so the
    weighted-memory matmul streams per m-tile with unnormalized exp weights and
    one final 1/S scale.
  - weighted-memory matmul runs on the fp32 memory tiles reinterpreted as
    float32r (full-rate fp32 datapath) with the exp-weight column stationary.
"""

import numpy as np

P = 128
BS, M, D, H = 64, 2048, 1024, 1024
NCORES = 8
BSL = BS // NCORES  # local batches per core
DK = D // P
HK = H // P
MTILE = 512
NMT = M // MTILE
MSUB = MTILE // P

_cache = {}


def _build_program():
    import concourse.bacc as bacc
    import concourse.tile as tile
    from concourse import mybir
    from concourse.masks import make_identity

    f32 = mybir.dt.float32
    f32r = mybir.dt.float32r
    bf16 = mybir.dt.bfloat16
    AF = mybir.ActivationFunctionType

    nc = bacc.Bacc(trn_type="TRN2", debug=False, target_bir_lowering=False)

    query = nc.dram_tensor("query", [BSL, D], f32, kind="ExternalInput").ap()
    memory = nc.dram_tensor("memory", [BSL, M, D], f32, kind="ExternalInput").ap()
    Wq = nc.dram_tensor("Wq", [H, D], f32, kind="ExternalInput").ap()
    bq = nc.dram_tensor("bq", [H], f32, kind="ExternalInput").ap()
    Wm = nc.dram_tensor("Wm", [H, D], f32, kind="ExternalInput").ap()
    v = nc.dram_tensor("v", [H], f32, kind="ExternalInput").ap()

    weights_o = nc.dram_tensor("weights", [BSL, 1, M], f32, kind="ExternalOutput").ap()
    wmem_o = nc.dram_tensor("wmem", [BSL, 1, D], f32, kind="ExternalOutput").ap()

    with tile.TileContext(nc) as tc:
        with (
            tc.tile_pool(name="persist", bufs=1) as pp,
            tc.tile_pool(name="psum_main", bufs=2, space="PSUM") as psum_main,
            tc.tile_pool(name="psum_attn", bufs=2, space="PSUM") as psum_attn,
            tc.tile_pool(name="psum_w", bufs=1, space="PSUM") as psum_w,
            tc.tile_pool(name="mem", bufs=3) as mempool,
            tc.tile_pool(name="memb", bufs=6) as membpool,
            tc.tile_pool(name="memt", bufs=2) as memtpool,
            tc.tile_pool(name="hid", bufs=2) as hidpool,
            tc.tile_pool(name="small", bufs=2) as spool,
        ):
            # ---- setup: transposed bf16 weights (WqT transient, WmT persistent) ----
            wmT = pp.tile([P, DK, H], bf16, tag="wmT")
            qpT = pp.tile([P, HK, BSL], f32, tag="qpT")
            bq128 = pp.tile([P, HK], f32, tag="bq128")
            v_bf = pp.tile([P, HK], bf16, tag="v_bf")
            ident16 = pp.tile([16, 16], f32, tag="ident16")
            make_identity(nc, ident16)

            with tc.tile_pool(name="setup", bufs=2) as sp, tc.tile_pool(
                name="setup1", bufs=1
            ) as sp1:
                wqT = sp1.tile([P, DK, H], bf16, tag="wqT")
                for target, wsrc in ((wqT, Wq), (wmT, Wm)):
                    for hk in range(HK):
                        cf = sp.tile([P, D], f32, tag="cf")
                        nc.sync.dma_start(cf, wsrc[hk * P : (hk + 1) * P, :])
                        cb = sp.tile([P, D], bf16, tag="cb")
                        nc.vector.tensor_copy(cb, cf)
                        nc.sync.dma_start_transpose(
                            target[:, :, hk * P : (hk + 1) * P], cb
                        )

                qT = sp1.tile([P, DK, BSL], f32, tag="qT")
                for o in range(DK):
                    nc.sync.dma_start(
                        qT[:, o, :],
                        query[:, o * P : (o + 1) * P].rearrange("b p -> p b"),
                    )
                qT_bf = sp1.tile([P, DK, BSL], bf16, tag="qT_bf")
                nc.vector.tensor_copy(qT_bf, qT)

                nc.sync.dma_start(bq128, bq.rearrange("(o p) -> p o", p=P))
                v128 = sp1.tile([P, HK], f32, tag="v128")
                nc.sync.dma_start(v128, v.rearrange("(o p) -> p o", p=P))
                nc.vector.tensor_copy(v_bf, v128)

                # q_projT[h, b] = sum_d WqT[d, h] * queryT[d, b]  (+ bq)
                for hk in range(HK):
                    psq = psum_main.tile([P, BSL], f32, tag="ph")
                    for dk in range(DK):
                        nc.tensor.matmul(
                            psq,
                            lhsT=wqT[:, dk, hk * P : (hk + 1) * P],
                            rhs=qT_bf[:, dk, :],
                            start=(dk == 0),
                            stop=(dk == DK - 1),
                        )
                    nc.scalar.activation(
                        qpT[:, hk, :], psq, AF.Identity, bias=bq128[:, hk : hk + 1]
                    )

            # ---- main loop over local batches ----
            for b in range(BSL):
                e_all = spool.tile([1, M], f32, tag="e_all")
                esum = spool.tile([1, NMT], f32, tag="esum")
                psw = psum_w.tile([1, D], f32, tag="pw")
                memb_tiles = []

                for mt in range(NMT):
                    mn = mempool.tile([P, MSUB, D], f32, tag="mn")
                    nc.sync.dma_start(
                        mn,
                        memory[b, mt * MTILE : (mt + 1) * MTILE, :].rearrange(
                            "(s p) d -> p s d", p=P
                        ),
                    )
                    mb_ = membpool.tile([P, MSUB, D], bf16, tag="mb")
                    nc.vector.tensor_copy(mb_, mn)
                    mT = memtpool.tile([P, DK, MTILE], bf16, tag="mT")
                    for s in range(MSUB):
                        nc.sync.dma_start_transpose(
                            mT[:, :, s * P : (s + 1) * P], mb_[:, s, :]
                        )

                    hid = hidpool.tile([P, HK, MTILE], bf16, tag="hid")
                    psa = psum_attn.tile([1, MTILE], f32, tag="pa")
                    for hk in range(HK):
                        psh = psum_main.tile([P, MTILE], f32, tag="ph")
                        for dk in range(DK):
                            nc.tensor.matmul(
                                psh,
                                lhsT=wmT[:, dk, hk * P : (hk + 1) * P],
                                rhs=mT[:, dk, :],
                                start=(dk == 0),
                                stop=(dk == DK - 1),
                            )
                        nc.scalar.activation(
                            hid[:, hk, :], psh, AF.Tanh, bias=qpT[:, hk, b : b + 1]
                        )
                        nc.tensor.matmul(
                            psa,
                            lhsT=v_bf[:, hk : hk + 1],
                            rhs=hid[:, hk, :],
                            start=(hk == 0),
                            stop=(hk == HK - 1),
                        )
                    nc.scalar.activation(
                        e_all[:, mt * MTILE : (mt + 1) * MTILE],
                        psa,
                        AF.Exp,
                        accum_out=esum[:, mt : mt + 1],
                    )
                    memb_tiles.append(mb_)

                # e [1, M] -> [P, M//P] so exp-weights can be a stationary column:
                # strided DMA to [M//P, P], then a tiny PE transpose.
                e_sp = spool.tile([M // P, P], f32, tag="e_sp")
                nc.sync.dma_start(e_sp, e_all.rearrange("a (q r) -> a q r", q=M // P))
                peT = psum_attn.tile([P, M // P], f32, tag="peT")
                nc.tensor.transpose(peT, e_sp, ident16)
                e128 = spool.tile([P, NMT * MSUB], bf16, tag="e128")
                nc.vector.tensor_copy(e128, peT)

                # wmem[d] = sum_m e[m] * memory[m, d]  (normalized by 1/S below)
                for mt in range(NMT):
                    mb_ = memb_tiles[mt]
                    for s in range(MSUB):
                        lhs = e128[:, mt * MSUB + s : mt * MSUB + s + 1]
                        for dh in range(2):
                            nc.tensor.matmul(
                                psw[:, dh * 512 : (dh + 1) * 512],
                                lhsT=lhs,
                                rhs=mb_[:, s, dh * 512 : (dh + 1) * 512],
                                start=(mt == 0 and s == 0),
                                stop=(mt == NMT - 1 and s == MSUB - 1),
                            )

                S = spool.tile([1, 1], f32, tag="S")
                nc.vector.tensor_reduce(
                    S, esum, axis=mybir.AxisListType.X, op=mybir.AluOpType.add
                )
                rinv = spool.tile([1, 1], f32, tag="rinv")
                nc.vector.reciprocal(rinv, S)

                nc.scalar.mul(e_all, e_all, rinv)
                nc.sync.dma_start(weights_o[b, 0, :], e_all)
                wm_sb = spool.tile([1, D], f32, tag="wm_sb")
                nc.scalar.mul(wm_sb, psw, rinv)
                nc.sync.dma_start(wmem_o[b, 0, :], wm_sb)

    nc.compile()
    return nc


def _get_program():
    if "nc" not in _cache:
        _cache["nc"] = _build_program()
    return _cache["nc"]


def _shard_inputs(query, memory, Wq, bq, Wm, v):
    in_maps = []
    for c in range(NCORES):
        sl = slice(c * BSL, (c + 1) * BSL)
        in_maps.append(
            {
                "query": np.ascontiguousarray(query[sl], dtype=np.float32),
                "memory": np.ascontiguousarray(memory[sl], dtype=np.float32),
                "Wq": np.asarray(Wq, dtype=np.float32),
                "bq": np.asarray(bq, dtype=np.float32),
                "Wm": np.asarray(Wm, dtype=np.float32),
                "v": np.asarray(v, dtype=np.float32),
            }
        )
    return in_maps


def kernel(query, memory, Wq, bq, Wm, v, _trace=False, _trace_kwargs=None):
    from concourse.bass_utils import run_bass_kernel_spmd

    nc = _get_program()
    in_maps = _shard_inputs(
        np.asarray(query), np.asarray(memory), np.asarray(Wq),
        np.asarray(bq), np.asarray(Wm), np.asarray(v),
    )
    res = run_bass_kernel_spmd(
        nc, in_maps, list(range(NCORES)), trace=_trace, **(_trace_kwargs or {})
    )
    weights = np.concatenate([res.results[c]["weights"] for c in range(NCORES)], axis=0)
    wmem = np.concatenate([res.results[c]["wmem"] for c in range(NCORES)], axis=0)
    if _trace:
        _cache["last_results"] = res
    return weights.astype(np.float32), wmem.astype(np.float32)


# revision 18
# speedup vs baseline: 126.1200x; 126.1200x over previous
"""Additive (Bahdanau) attention kernel for Trainium2, data-parallel over 8 NeuronCores.

Computation (per batch b):
    q_proj = query @ Wq.T + bq                  # [H]
    m_proj = memory[b] @ Wm.T                   # [M, H]
    hidden = tanh(q_proj + m_proj)              # [M, H]
    attn   = hidden @ v                         # [M]
    w      = softmax(attn)                      # [M]
    wmem   = w @ memory[b]                      # [D]

Layout strategy (per core, 8 local batches):
  - m_proj computed transposed: [h on partitions, m free], so the q_proj add +
    tanh fuse into one ACT instruction (per-partition bias), and the v-reduction
    is a PE matmul with v as the (1-column) stationary operand.
  - memory tiles are cast to bf16 and transposed to d-major via the DMA XBAR
    (fp32 has no XBAR path; bf16 matmul streams at 1 cycle/row vs 4 for fp32).
  - softmax without max-subtraction (attn is bounded by sum|v| ~ 25), so the
    weighted-memory matmul streams per m-tile with unnormalized exp weights and
    one final 1/S scale.
  - weighted-memory matmul runs on the fp32 memory tiles reinterpreted as
    float32r (full-rate fp32 datapath) with the exp-weight column stationary.
"""

import numpy as np

P = 128
BS, M, D, H = 64, 2048, 1024, 1024
NCORES = 8
BSL = BS // NCORES  # local batches per core
DK = D // P
HK = H // P
MTILE = 512
NMT = M // MTILE
MSUB = MTILE // P

_cache = {}


def _build_program(repeat=1):
    import concourse.bacc as bacc
    import concourse.tile as tile
    from concourse import mybir
    from concourse.masks import make_identity

    f32 = mybir.dt.float32
    f32r = mybir.dt.float32r
    bf16 = mybir.dt.bfloat16
    AF = mybir.ActivationFunctionType

    nc = bacc.Bacc(trn_type="TRN2", debug=False, target_bir_lowering=False)

    query = nc.dram_tensor("query", [BSL, D], f32, kind="ExternalInput").ap()
    memory = nc.dram_tensor("memory", [BSL, M, D], f32, kind="ExternalInput").ap()
    Wq = nc.dram_tensor("Wq", [H, D], f32, kind="ExternalInput").ap()
    bq = nc.dram_tensor("bq", [H], f32, kind="ExternalInput").ap()
    Wm = nc.dram_tensor("Wm", [H, D], f32, kind="ExternalInput").ap()
    v = nc.dram_tensor("v", [H], f32, kind="ExternalInput").ap()

    weights_o = nc.dram_tensor("weights", [BSL, 1, M], f32, kind="ExternalOutput").ap()
    wmem_o = nc.dram_tensor("wmem", [BSL, 1, D], f32, kind="ExternalOutput").ap()

    with tile.TileContext(nc) as tc:
        with (
            tc.tile_pool(name="persist", bufs=1) as pp,
            tc.tile_pool(name="psum_main", bufs=2, space="PSUM") as psum_main,
            tc.tile_pool(name="psum_attn", bufs=2, space="PSUM") as psum_attn,
            tc.tile_pool(name="psum_w", bufs=1, space="PSUM") as psum_w,
            tc.tile_pool(name="mem", bufs=3) as mempool,
            tc.tile_pool(name="memb", bufs=6) as membpool,
            tc.tile_pool(name="memt", bufs=2) as memtpool,
            tc.tile_pool(name="hid", bufs=2) as hidpool,
            tc.tile_pool(name="small", bufs=2) as spool,
        ):
            # ---- setup: transposed bf16 weights (WqT transient, WmT persistent) ----
            wmT = pp.tile([P, DK, H], bf16, tag="wmT")
            qpT = pp.tile([P, HK, BSL], f32, tag="qpT")
            bq128 = pp.tile([P, HK], f32, tag="bq128")
            v_bf = pp.tile([P, HK], bf16, tag="v_bf")
            ident16 = pp.tile([16, 16], f32, tag="ident16")
            make_identity(nc, ident16)

            with tc.tile_pool(name="setup", bufs=2) as sp, tc.tile_pool(
                name="setup1", bufs=1
            ) as sp1:
                wqT = sp1.tile([P, DK, H], bf16, tag="wqT")
                for target, wsrc in ((wqT, Wq), (wmT, Wm)):
                    for hk in range(HK):
                        cf = sp.tile([P, D], f32, tag="cf")
                        nc.sync.dma_start(cf, wsrc[hk * P : (hk + 1) * P, :])
                        cb = sp.tile([P, D], bf16, tag="cb")
                        nc.vector.tensor_copy(cb, cf)
                        nc.sync.dma_start_transpose(
                            target[:, :, hk * P : (hk + 1) * P], cb
                        )

                qT = sp1.tile([P, DK, BSL], f32, tag="qT")
                for o in range(DK):
                    nc.sync.dma_start(
                        qT[:, o, :],
                        query[:, o * P : (o + 1) * P].rearrange("b p -> p b"),
                    )
                qT_bf = sp1.tile([P, DK, BSL], bf16, tag="qT_bf")
                nc.vector.tensor_copy(qT_bf, qT)

                nc.sync.dma_start(bq128, bq.rearrange("(o p) -> p o", p=P))
                v128 = sp1.tile([P, HK], f32, tag="v128")
                nc.sync.dma_start(v128, v.rearrange("(o p) -> p o", p=P))
                nc.vector.tensor_copy(v_bf, v128)

                # q_projT[h, b] = sum_d WqT[d, h] * queryT[d, b]  (+ bq)
                for hk in range(HK):
                    psq = psum_main.tile([P, BSL], f32, tag="ph")
                    for dk in range(DK):
                        nc.tensor.matmul(
                            psq,
                            lhsT=wqT[:, dk, hk * P : (hk + 1) * P],
                            rhs=qT_bf[:, dk, :],
                            start=(dk == 0),
                            stop=(dk == DK - 1),
                        )
                    nc.scalar.activation(
                        qpT[:, hk, :], psq, AF.Identity, bias=bq128[:, hk : hk + 1]
                    )

            # ---- main loop over local batches ----
            for b in [b for _ in range(repeat) for b in range(BSL)]:
                e_all = spool.tile([1, M], f32, tag="e_all")
                esum = spool.tile([1, NMT], f32, tag="esum")
                psw = psum_w.tile([1, D], f32, tag="pw")
                memb_tiles = []

                for mt in range(NMT):
                    mn = mempool.tile([P, MSUB, D], f32, tag="mn")
                    nc.sync.dma_start(
                        mn,
                        memory[b, mt * MTILE : (mt + 1) * MTILE, :].rearrange(
                            "(s p) d -> p s d", p=P
                        ),
                    )
                    mb_ = membpool.tile([P, MSUB, D], bf16, tag="mb")
                    nc.vector.tensor_copy(mb_, mn)
                    mT = memtpool.tile([P, DK, MTILE], bf16, tag="mT")
                    for s in range(MSUB):
                        nc.sync.dma_start_transpose(
                            mT[:, :, s * P : (s + 1) * P], mb_[:, s, :]
                        )

                    hid = hidpool.tile([P, HK, MTILE], bf16, tag="hid")
                    psa = psum_attn.tile([1, MTILE], f32, tag="pa")
                    for hk in range(HK):
                        psh = psum_main.tile([P, MTILE], f32, tag="ph")
                        for dk in range(DK):
                            nc.tensor.matmul(
                                psh,
                                lhsT=wmT[:, dk, hk * P : (hk + 1) * P],
                                rhs=mT[:, dk, :],
                                start=(dk == 0),
                                stop=(dk == DK - 1),
                            )
                        nc.scalar.activation(
                            hid[:, hk, :], psh, AF.Tanh, bias=qpT[:, hk, b : b + 1]
                        )
                        nc.tensor.matmul(
                            psa,
                            lhsT=v_bf[:, hk : hk + 1],
                            rhs=hid[:, hk, :],
                            start=(hk == 0),
                            stop=(hk == HK - 1),
                        )
                    nc.scalar.activation(
                        e_all[:, mt * MTILE : (mt + 1) * MTILE],
                        psa,
                        AF.Exp,
                        accum_out=esum[:, mt : mt + 1],
                    )
                    memb_tiles.append(mb_)

                # e [1, M] -> [P, M//P] so exp-weights can be a stationary column:
                # strided DMA to [M//P, P], then a tiny PE transpose.
                e_sp = spool.tile([M // P, P], f32, tag="e_sp")
                nc.sync.dma_start(e_sp, e_all.rearrange("a (q r) -> a q r", q=M // P))
                peT = psum_attn.tile([P, M // P], f32, tag="peT")
                nc.tensor.transpose(peT, e_sp, ident16)
                e128 = spool.tile([P, NMT * MSUB], bf16, tag="e128")
                nc.vector.tensor_copy(e128, peT)

                # wmem[d] = sum_m e[m] * memory[m, d]  (normalized by 1/S below)
                for mt in range(NMT):
                    mb_ = memb_tiles[mt]
                    for s in range(MSUB):
                        lhs = e128[:, mt * MSUB + s : mt * MSUB + s + 1]
                        for dh in range(2):
                            nc.tensor.matmul(
                                psw[:, dh * 512 : (dh + 1) * 512],
                                lhsT=lhs,
                                rhs=mb_[:, s, dh * 512 : (dh + 1) * 512],
                                start=(mt == 0 and s == 0),
                                stop=(mt == NMT - 1 and s == MSUB - 1),
                            )

                S = spool.tile([1, 1], f32, tag="S")
                nc.vector.tensor_reduce(
                    S, esum, axis=mybir.AxisListType.X, op=mybir.AluOpType.add
                )
                rinv = spool.tile([1, 1], f32, tag="rinv")
                nc.vector.reciprocal(rinv, S)

                nc.scalar.mul(e_all, e_all, rinv)
                nc.sync.dma_start(weights_o[b, 0, :], e_all)
                wm_sb = spool.tile([1, D], f32, tag="wm_sb")
                nc.scalar.mul(wm_sb, psw, rinv)
                nc.sync.dma_start(wmem_o[b, 0, :], wm_sb)

    nc.compile()
    return nc


def _get_program(repeat=1):
    key = ("nc", repeat)
    if key not in _cache:
        _cache[key] = _build_program(repeat)
    return _cache[key]


def _shard_inputs(query, memory, Wq, bq, Wm, v):
    in_maps = []
    for c in range(NCORES):
        sl = slice(c * BSL, (c + 1) * BSL)
        in_maps.append(
            {
                "query": np.ascontiguousarray(query[sl], dtype=np.float32),
                "memory": np.ascontiguousarray(memory[sl], dtype=np.float32),
                "Wq": np.asarray(Wq, dtype=np.float32),
                "bq": np.asarray(bq, dtype=np.float32),
                "Wm": np.asarray(Wm, dtype=np.float32),
                "v": np.asarray(v, dtype=np.float32),
            }
        )
    return in_maps


def kernel(query, memory, Wq, bq, Wm, v, _trace=False, _trace_kwargs=None):
    from concourse.bass_utils import run_bass_kernel_spmd

    nc = _get_program()
    in_maps = _shard_inputs(
        np.asarray(query), np.asarray(memory), np.asarray(Wq),
        np.asarray(bq), np.asarray(Wm), np.asarray(v),
    )
    res = run_bass_kernel_spmd(
        nc, in_maps, list(range(NCORES)), trace=_trace, **(_trace_kwargs or {})
    )
    weights = np.concatenate([res.results[c]["weights"] for c in range(NCORES)], axis=0)
    wmem = np.concatenate([res.results[c]["wmem"] for c in range(NCORES)], axis=0)
    if _trace:
        _cache["last_results"] = res
    return weights.astype(np.float32), wmem.astype(np.float32)


# revision 55
# speedup vs baseline: 134.9951x; 1.0704x over previous
"""Additive (Bahdanau) attention kernel for Trainium2, data-parallel over 8 NeuronCores.

Computation (per batch b):
    q_proj = query @ Wq.T + bq                  # [H]
    m_proj = memory[b] @ Wm.T                   # [M, H]
    hidden = tanh(q_proj + m_proj)              # [M, H]
    attn   = hidden @ v                         # [M]
    w      = softmax(attn)                      # [M]
    wmem   = w @ memory[b]                      # [D]

Layout strategy (per core, 8 local batches):
  - m_proj computed transposed: [h on partitions, m free], so the q_proj add +
    tanh fuse into one ACT instruction (per-partition bias), and the v-reduction
    is a PE matmul with v as the (1-column) stationary operand.
  - memory tiles are cast to bf16 and transposed to d-major via the DMA XBAR
    (fp32 has no XBAR path; bf16 matmul streams at 1 cycle/row vs 4 for fp32).
  - softmax without max-subtraction (attn is bounded by sum|v| ~ 25), so the
    weighted-memory matmul streams per m-tile with unnormalized exp weights and
    one final 1/S scale.
  - weighted-memory matmul runs on the fp32 memory tiles reinterpreted as
    float32r (full-rate fp32 datapath) with the exp-weight column stationary.
"""

import os

import numpy as np

USE_TTR = os.environ.get("K_TTR", "1") == "1"
USE_CAST_DMA = os.environ.get("K_CASTDMA", "1") == "1"

P = 128
BS, M, D, H = 64, 2048, 1024, 1024
NCORES = 8
BSL = BS // NCORES  # local batches per core
DK = D // P
HK = H // P
MTILE = 512
NMT = M // MTILE
MSUB = MTILE // P

_cache = {}


def _build_program(repeat=1):
    import concourse.bacc as bacc
    import concourse.tile as tile
    from concourse import mybir
    from concourse.masks import make_identity

    f32 = mybir.dt.float32
    f32r = mybir.dt.float32r
    bf16 = mybir.dt.bfloat16
    AF = mybir.ActivationFunctionType

    nc = bacc.Bacc(trn_type="TRN2", debug=False, target_bir_lowering=False)

    query = nc.dram_tensor("query", [BSL, D], f32, kind="ExternalInput").ap()
    memory = nc.dram_tensor("memory", [BSL, M, D], f32, kind="ExternalInput").ap()
    Wq = nc.dram_tensor("Wq", [H, D], f32, kind="ExternalInput").ap()
    bq = nc.dram_tensor("bq", [H], f32, kind="ExternalInput").ap()
    Wm = nc.dram_tensor("Wm", [H, D], f32, kind="ExternalInput").ap()
    v = nc.dram_tensor("v", [H], f32, kind="ExternalInput").ap()

    weights_o = nc.dram_tensor("weights", [BSL, 1, M], f32, kind="ExternalOutput").ap()
    wmem_o = nc.dram_tensor("wmem", [BSL, 1, D], f32, kind="ExternalOutput").ap()

    with tile.TileContext(nc) as tc:
        with (
            tc.tile_pool(name="persist", bufs=1) as pp,
            tc.tile_pool(name="psum_main", bufs=4, space="PSUM") as psum_main,
            tc.tile_pool(name="psum_attn", bufs=2, space="PSUM") as psum_attn,
            tc.tile_pool(name="memb", bufs=4) as membpool,
            tc.tile_pool(name="memt", bufs=6) as memtpool,
            tc.tile_pool(name="hid", bufs=2) as hidpool,
            tc.tile_pool(name="small", bufs=2) as spool,
        ):
            # ---- setup: transposed bf16 weights (WqT transient, WmT persistent) ----
            # Wm goes FIRST (its transposes gate the first main matmul); wmT is
            # split per h-chunk so matmuls start as soon as their chunk lands.
            # Big loads go on gpsimd (SWDGE) so the SP HWDGE queue carries only
            # XBAR transposes (no head-of-line blocking behind cast waits).
            wmT = [
                pp.tile([P, DK, P], bf16, tag=f"wmT{hk}", name=f"wmT{hk}")
                for hk in range(HK)
            ]
            qpT = pp.tile([P, HK, BSL], f32, tag="qpT")
            bq128 = pp.tile([P, HK], f32, tag="bq128")
            v_bf = pp.tile([P, HK], bf16, tag="v_bf")
            vrep = pp.tile([P, HK, P], bf16, tag="vrep")
            ident128 = pp.tile([P, P], f32, tag="ident128")
            make_identity(nc, ident128)
            ident128b = pp.tile([P, P], bf16, tag="ident128b")
            nc.vector.tensor_copy(ident128b, ident128)

            def emit_weight_transpose(target, wsrc):
                # gpsimd (SWDGE) DMA casts fp32->bf16 in flight: no fp32
                # staging tile, no DVE cast op.
                for half in range(2):
                    cb = membpool.tile([P, MSUB, D], bf16, tag="mb", name="mb_w")
                    nc.gpsimd.dma_start(
                        cb,
                        wsrc[half * 512 : (half + 1) * 512, :].rearrange(
                            "(s p) d -> p s d", p=P
                        ),
                    )
                    for j in range(4):
                        nc.sync.dma_start_transpose(target[half * 4 + j], cb[:, j, :])

            def emit_mem_stage(b, mt):
                mb_ = membpool.tile([P, MSUB, D], bf16, tag="mb", name="mb_")
                nc.gpsimd.dma_start(
                    mb_,
                    memory[b, mt * MTILE : (mt + 1) * MTILE, :].rearrange(
                        "(s p) d -> p s d", p=P
                    ),
                )
                mT = memtpool.tile([P, DK, MTILE], bf16, tag="mT", name="mT")
                for s in range(MSUB):
                    nc.sync.dma_start_transpose(
                        mT[:, :, s * P : (s + 1) * P], mb_[:, s, :]
                    )
                return mb_, mT

            def emit_mem_stage_pe(b, mt):
                # PE-transpose variant for startup tiles: the PE is idle during
                # the prefix, while the DMA lane is the startup bottleneck.
                mb_ = membpool.tile([P, MSUB, D], bf16, tag="mb", name="mb_")
                nc.gpsimd.dma_start(
                    mb_,
                    memory[b, mt * MTILE : (mt + 1) * MTILE, :].rearrange(
                        "(s p) d -> p s d", p=P
                    ),
                )
                mT = memtpool.tile([P, DK, MTILE], bf16, tag="mT", name="mT")
                for s in range(MSUB):
                    for dk in range(DK):
                        psT = psum_main.tile([P, P], bf16, tag="ph", name="psT")
                        nc.tensor.transpose(
                            psT, mb_[:, s, dk * P : (dk + 1) * P], ident128b
                        )
                        nc.vector.tensor_copy(
                            mT[:, dk, s * P : (s + 1) * P], psT
                        )
                return mb_, mT

            prefetched = {}
            with tc.tile_pool(name="setup1", bufs=1) as sp1:
                wqT = [
                    sp1.tile([P, DK, P], bf16, tag=f"wqT{hk}", name=f"wqT{hk}")
                    for hk in range(HK)
                ]
                emit_weight_transpose(wmT, Wm)
                prefetched[(0, 0)] = emit_mem_stage_pe(0, 0)
                prefetched[(0, 1)] = emit_mem_stage_pe(0, 1)
                emit_weight_transpose(wqT, Wq)

                qT_bf = sp1.tile([P, DK, BSL], bf16, tag="qT_bf")
                for o in range(DK):
                    nc.gpsimd.dma_start(
                        qT_bf[:, o, :],
                        query[:, o * P : (o + 1) * P].rearrange("b p -> p b"),
                    )
                nc.gpsimd.dma_start(bq128, bq.rearrange("(o p) -> p o", p=P))
                nc.gpsimd.dma_start(v_bf, v.rearrange("(o p) -> p o", p=P))
                for hk in range(HK):
                    nc.vector.tensor_copy(
                        vrep[:, hk, :], v_bf[:, hk : hk + 1].to_broadcast((P, P))
                    )

                # q_projT[h, b] = sum_d WqT[d, h] * queryT[d, b]  (+ bq)
                for hk in range(HK):
                    psq = psum_attn.tile([P, BSL], f32, tag="pa")
                    for dk in range(DK):
                        nc.tensor.matmul(
                            psq,
                            lhsT=wqT[hk][:, dk, :],
                            rhs=qT_bf[:, dk, :],
                            start=(dk == 0),
                            stop=(dk == DK - 1),
                        )
                    nc.scalar.activation(
                        qpT[:, hk, :], psq, AF.Identity, bias=bq128[:, hk : hk + 1]
                    )

                for mt in range(2, NMT):
                    prefetched[(0, mt)] = emit_mem_stage(0, mt)

            # ---- main loop over local batches ----
            for it, b in enumerate(
                [b for _ in range(repeat) for b in range(BSL)]
            ):
                esum4 = spool.tile([P, NMT], f32, tag="esum4")
                wacc = spool.tile([P, DK], f32, tag="wacc")
                ebcs = []

                for mt in range(NMT):
                    if it == 0 and (b, mt) in prefetched:
                        mb_, mT = prefetched.pop((b, mt))
                    else:
                        mb_, mT = emit_mem_stage(b, mt)

                    hid = hidpool.tile([P, HK, MTILE], bf16, tag="hid")
                    psa = psum_attn.tile([P, MTILE], f32, tag="pa")
                    for hk in range(HK):
                        psh = psum_main.tile([P, MTILE], f32, tag="ph")
                        for dk in range(DK):
                            nc.tensor.matmul(
                                psh,
                                lhsT=wmT[hk][:, dk, :],
                                rhs=mT[:, dk, :],
                                start=(dk == 0),
                                stop=(dk == DK - 1),
                            )
                        nc.scalar.activation(
                            hid[:, hk, :], psh, AF.Tanh, bias=qpT[:, hk, b : b + 1]
                        )
                        nc.tensor.matmul(
                            psa,
                            lhsT=vrep[:, hk, :],
                            rhs=hid[:, hk, :],
                            start=(hk == 0),
                            stop=(hk == HK - 1),
                        )
                    ebc = spool.tile([P, MTILE], f32, tag="ebc", bufs=6)
                    nc.scalar.activation(
                        ebc, psa, AF.Exp, accum_out=esum4[:, mt : mt + 1]
                    )
                    ebcs.append(ebc)

                    # wmem partial on DVE: wacc[d-part, dk] += sum_m e[m]*memT[d, dk, m]
                    junk = spool.tile([P, DK, MTILE], bf16, tag="junk")
                    nc.vector.tensor_tensor(
                        junk,
                        mT,
                        ebc.unsqueeze(1).to_broadcast((P, DK, MTILE)),
                        mybir.AluOpType.mult,
                    )
                    wp = spool.tile([P, DK], f32, tag="wp", bufs=3)
                    nc.vector.tensor_reduce(
                        wp, junk, axis=mybir.AxisListType.X, op=mybir.AluOpType.add
                    )
                    if mt == 0:
                        nc.vector.tensor_copy(wacc, wp)
                    else:
                        nc.vector.tensor_add(wacc, wacc, wp)

                # S is partition-replicated by construction (every psum row of
                # attn got the same values), so 1/S needs no broadcast.
                S128 = spool.tile([P, 1], f32, tag="S128")
                nc.vector.tensor_reduce(
                    S128, esum4, axis=mybir.AxisListType.X, op=mybir.AluOpType.add
                )
                rinv128 = spool.tile([P, 1], f32, tag="rinv128")
                nc.vector.reciprocal(rinv128, S128)

                wout = spool.tile([1, M], f32, tag="wout")
                for mt in range(NMT):
                    nc.vector.tensor_scalar_mul(
                        wout[:, mt * MTILE : (mt + 1) * MTILE],
                        ebcs[mt][0:1, :],
                        rinv128[0:1, :],
                    )
                nc.gpsimd.dma_start(weights_o[b, 0, :], wout)

                # normalize wacc, transpose [128, DK] -> [DK, 128], write out
                waccs = spool.tile([P, DK], f32, tag="waccs")
                nc.vector.tensor_scalar_mul(waccs, wacc, rinv128)
                pwT = psum_attn.tile([DK, P], f32, tag="pa", name="pwT")
                nc.tensor.transpose(pwT, waccs, ident128)
                wm_sb = spool.tile([DK, P], f32, tag="wm_sb")
                nc.vector.tensor_copy(wm_sb, pwT)
                nc.gpsimd.dma_start(wmem_o[b, 0, :], wm_sb)

    nc.compile()
    return nc


def _get_program(repeat=1):
    key = ("nc", repeat)
    if key not in _cache:
        _cache[key] = _build_program(repeat)
    return _cache[key]


def _shard_inputs(query, memory, Wq, bq, Wm, v):
    in_maps = []
    for c in range(NCORES):
        sl = slice(c * BSL, (c + 1) * BSL)
        in_maps.append(
            {
                "query": np.ascontiguousarray(query[sl], dtype=np.float32),
                "memory": np.ascontiguousarray(memory[sl], dtype=np.float32),
                "Wq": np.asarray(Wq, dtype=np.float32),
                "bq": np.asarray(bq, dtype=np.float32),
                "Wm": np.asarray(Wm, dtype=np.float32),
                "v": np.asarray(v, dtype=np.float32),
            }
        )
    return in_maps


def kernel(query, memory, Wq, bq, Wm, v, _trace=False, _trace_kwargs=None):
    from concourse.bass_utils import run_bass_kernel_spmd

    nc = _get_program()
    in_maps = _shard_inputs(
        np.asarray(query), np.asarray(memory), np.asarray(Wq),
        np.asarray(bq), np.asarray(Wm), np.asarray(v),
    )
    res = run_bass_kernel_spmd(
        nc, in_maps, list(range(NCORES)), trace=_trace, **(_trace_kwargs or {})
    )
    weights = np.concatenate([res.results[c]["weights"] for c in range(NCORES)], axis=0)
    wmem = np.concatenate([res.results[c]["wmem"] for c in range(NCORES)], axis=0)
    if _trace:
        _cache["last_results"] = res
    return weights.astype(np.float32), wmem.astype(np.float32)
